# revision 17
# baseline (speedup 1.0000x reference)
"""Distributed GNN message-passing kernel for 8 Trainium2 NeuronCores.

Pipeline (mathematically equal to the reference, using G@(X@W) == (G@X)@W):
  u1 = G @ embed            (SpMM: gather + selection-matrix matmuls on PE)
  h1 = LN(relu(u1 @ W1 + deg*b1))
  H1 = AllGather(h1)        (the only collective)
  u2 = G @ H1
  h2 = LN(relu(u2 @ W2 + deg*b2))
  emb = h2 + h1
  out[b] = emb[x[b]-1] for valid queries (owner-routed, reassembled on host)

Sharding: nodes (rows of G / dest rows) are split contiguously across the 8
cores; [D,D] weights replicated; embed replicated (input staging, free);
h1 exchanged with one AllGather.
"""
import sys

sys.path.insert(0, "/opt/trn_rl_repo")

import numpy as np
from concourse import bacc, bass, mybir, tile
from concourse.bass_utils import run_bass_kernel_spmd
from concourse.masks import make_identity

C = 8          # cores
P = 128        # partitions
GB = 8         # SpMM gather groups per indirect-DMA call
GB2 = 8        # final-gather rows-per-partition per indirect-DMA call
EPS = 1e-5

F32 = mybir.dt.float32
I32 = mybir.dt.int32


def _preprocess(x, embed, G_rows, G_cols, G_vals):
    """Host-side index preprocessing. Returns per-core in_map arrays + meta."""
    N, D = embed.shape
    NM = N // C
    assert N % C == 0
    T = (NM + P - 1) // P
    x = np.asarray(x).astype(np.int64)
    G_rows = np.asarray(G_rows).astype(np.int64)
    G_cols = np.asarray(G_cols).astype(np.int64)
    G_vals = np.asarray(G_vals).astype(np.float32)

    owner = G_rows // NM
    percore = []
    cnts = np.zeros((C, T), np.int64)
    for m in range(C):
        sel = np.where(owner == m)[0]
        ld = G_rows[sel] - m * NM
        order = np.argsort(ld, kind="stable")
        sel, ld = sel[order], ld[order]
        percore.append((sel, ld))
        cnts[m] = np.bincount(ld // P, minlength=T)

    gt = np.maximum(1, -(-cnts // P)).max(axis=0).astype(np.int64)  # [T] common
    goff = np.concatenate([[0], np.cumsum(gt)]).astype(np.int64)
    Gtot = int(goff[-1])
    NB = -(-Gtot // GB)
    Gpad = NB * GB

    gg = np.arange(Gtot * P) // P
    pp = np.arange(Gtot * P) % P

    S_imgs, idx_alls, degs = [], [], []
    for m in range(C):
        sel, ld = percore[m]
        tt = ld // P
        starts = np.concatenate([[0], np.cumsum(cnts[m])])
        r = np.arange(len(ld)) - starts[tt]
        slot = goff[tt] * P + r
        cols_pad = np.zeros(Gpad * P, np.int32)
        vals_pad = np.zeros(Gtot * P, np.float32)
        dof_pad = np.zeros(Gtot * P, np.int64)
        cols_pad[slot] = G_cols[sel]
        vals_pad[slot] = G_vals[sel]
        dof_pad[slot] = ld - tt * P
        S = np.zeros((Gtot, P, P), np.float32)
        S[gg, pp, dof_pad] = vals_pad
        S_imgs.append(np.ascontiguousarray(S.transpose(1, 0, 2).reshape(P, Gtot * P)))
        idx_alls.append(np.ascontiguousarray(cols_pad.reshape(Gpad, P).T))
        deg = np.zeros(T * P, np.float32)
        deg[:NM] = np.bincount(ld, weights=G_vals[sel], minlength=NM)
        degs.append(deg[None, :])

    # final query routing
    selq = (x >= 1) & (x < N + 1)
    idx = np.clip(x - 1, 0, N - 1)
    qowner = idx // NM
    rows_list, lidx_list = [], []
    for m in range(C):
        rows = np.where(selq & (qowner == m))[0]
        rows_list.append(rows)
        lidx_list.append((idx[rows] - m * NM).astype(np.int32))
    max_cnt = max(len(r) for r in rows_list)
    NFB = max(1, -(-max_cnt // (P * GB2)))
    cnt_pad = NFB * P * GB2
    lidx_alls = []
    for m in range(C):
        lp = np.zeros(cnt_pad, np.int32)
        lp[: len(lidx_list[m])] = lidx_list[m]
        lidx_alls.append(np.ascontiguousarray(lp.reshape(NFB * GB2, P).T))

    meta = dict(
        N=N, D=D, NM=NM, T=T, gt=gt, goff=goff, Gtot=Gtot, NB=NB, NFB=NFB,
        cnt_pad=cnt_pad, rows_list=rows_list,
    )
    arrays = dict(S_imgs=S_imgs, idx_alls=idx_alls, degs=degs,
                  lidx_alls=lidx_alls)
    return meta, arrays


def _build(meta, trivial_ln, debug=False):
    """Build the SPMD Bass program (identical across cores)."""
    N, D, T = meta["N"], meta["D"], meta["T"]
    gt, goff, Gtot, NB, NFB = (
        meta["gt"], meta["goff"], meta["Gtot"], meta["NB"], meta["NFB"])
    cnt_pad = meta["cnt_pad"]
    KD = D // P  # contraction chunks for dense matmul (4 for D=512)

    nc = bacc.Bacc(None, num_devices=C)

    # --- I/O ---
    embed_t = nc.dram_tensor("embed", [N, D], F32, kind="ExternalInput")
    S_t = nc.dram_tensor("S_img", [P, Gtot * P], F32, kind="ExternalInput")
    idx_t = nc.dram_tensor("idx_all", [P, NB * GB], I32, kind="ExternalInput")
    deg_t = nc.dram_tensor("deg", [1, T * P], F32, kind="ExternalInput")
    W1_t = nc.dram_tensor("W1r", [P, KD, D], F32, kind="ExternalInput")
    W2_t = nc.dram_tensor("W2r", [P, KD, D], F32, kind="ExternalInput")
    b1_t = nc.dram_tensor("b1", [1, D], F32, kind="ExternalInput")
    b2_t = nc.dram_tensor("b2", [1, D], F32, kind="ExternalInput")
    lidx_t = nc.dram_tensor("lidx_all", [P, NFB * GB2], I32, kind="ExternalInput")
    if not trivial_ln:
        gb_t = nc.dram_tensor("gb", [P, 4, D], F32, kind="ExternalInput")
    out_t = nc.dram_tensor("out_packed", [cnt_pad, D], F32, kind="ExternalOutput")
    if debug:
        dbg_u1 = nc.dram_tensor("dbg_u1", [T * P, D], F32, kind="ExternalOutput")
        dbg_h1 = nc.dram_tensor("dbg_h1", [T * P, D], F32, kind="ExternalOutput")
        dbg_H1f = nc.dram_tensor("dbg_H1f", [N, D], F32, kind="ExternalOutput")
        dbg_zg = nc.dram_tensor("dbg_zg", [P, GB * D], F32, kind="ExternalOutput")

    with tile.TileContext(nc) as tc:
        with (
            tc.tile_pool(name="zg", bufs=24) as zg_pool,
            tc.tile_pool(name="sp", bufs=3) as s_pool,
            tc.tile_pool(name="small", bufs=1) as small,
            tc.tile_pool(name="work", bufs=4) as work,
            tc.tile_pool(name="stat", bufs=6) as stat,
            tc.tile_pool(name="pu", bufs=2, space="PSUM") as psum_u,
            tc.tile_pool(name="pt", bufs=2, space="PSUM") as psum_t,
            tc.tile_pool(name="ph", bufs=2, space="PSUM") as psum_h,
            tc.tile_pool(name="dram", bufs=1, space="DRAM") as dram,
        ):
            # --- internal DRAM ---
            h1loc = dram.tile([T * P, D], F32)
            H1full = dram.tile([N, D], F32)
            embloc = dram.tile([T * P, D], F32)

            # --- constants ---
            W1_sb = small.tile([P, KD, D], F32)
            nc.sync.dma_start(out=W1_sb[:], in_=W1_t[:])
            W2_sb = small.tile([P, KD, D], F32)
            nc.sync.dma_start(out=W2_sb[:], in_=W2_t[:])
            b1_sb = small.tile([1, D], F32)
            nc.sync.dma_start(out=b1_sb[:], in_=b1_t[:])
            b2_sb = small.tile([1, D], F32)
            nc.sync.dma_start(out=b2_sb[:], in_=b2_t[:])
            deg_sb = small.tile([1, T * P], F32)
            nc.sync.dma_start(out=deg_sb[:], in_=deg_t[:])
            idx_sb = small.tile([P, NB * GB], I32)
            nc.sync.dma_start(out=idx_sb[:], in_=idx_t[:])
            lidx_sb = small.tile([P, NFB * GB2], I32)
            nc.sync.dma_start(out=lidx_sb[:], in_=lidx_t[:])
            ident = small.tile([P, P], F32)
            make_identity(nc, ident[:])
            eps_sb = small.tile([P, 1], F32)
            nc.vector.memset(eps_sb[:], EPS)
            if not trivial_ln:
                gb_sb = small.tile([P, 4, D], F32)
                nc.sync.dma_start(out=gb_sb[:], in_=gb_t[:])

            def layer(table_ap, W_sb, b_sb, gamma_i, h_out_dram, second):
                # gather producers: one [P,1]-indexed indirect DMA per group
                # (HW honors exactly one index per partition per call)
                zg_tiles = []
                for g in range(Gtot):
                    zg = zg_pool.tile([P, D], F32, tag="zg")
                    nc.gpsimd.indirect_dma_start(
                        out=zg[:],
                        out_offset=None,
                        in_=table_ap,
                        in_offset=bass.IndirectOffsetOnAxis(
                            ap=idx_sb[:, g : g + 1], axis=0
                        ),
                    )
                    zg_tiles.append(zg)

                for t in range(T):
                    g0, g1 = int(goff[t]), int(goff[t + 1])
                    ngt = g1 - g0
                    s_sb = s_pool.tile([P, ngt * P], F32, tag="s")
                    nc.sync.dma_start(
                        out=s_sb[:], in_=S_t[:, g0 * P : g1 * P]
                    )
                    pu = psum_u.tile([P, D], F32)
                    for gl in range(ngt):
                        nc.tensor.matmul(
                            out=pu[:],
                            lhsT=s_sb[:, gl * P : (gl + 1) * P],
                            rhs=zg_tiles[g0 + gl][:],
                            start=(gl == 0),
                            stop=(gl == ngt - 1),
                        )
                    u_sb = work.tile([P, D], F32, tag="u")
                    nc.vector.tensor_copy(out=u_sb[:], in_=pu[:])
                    if debug and not second:
                        nc.sync.dma_start(
                            out=dbg_u1[t * P : (t + 1) * P, :], in_=u_sb[:]
                        )
                    pt = psum_t.tile([P, D], F32)
                    for k in range(KD):
                        nc.tensor.transpose(
                            out=pt[:, k * P : (k + 1) * P],
                            in_=u_sb[:, k * P : (k + 1) * P],
                            identity=ident[:],
                        )
                    uT_sb = work.tile([P, D], F32, tag="ut")
                    nc.vector.tensor_copy(out=uT_sb[:], in_=pt[:])
                    ph = psum_h.tile([P, D], F32)
                    for k in range(KD):
                        nc.tensor.matmul(
                            out=ph[:],
                            lhsT=uT_sb[:, k * P : (k + 1) * P],
                            rhs=W_sb[:, k, :],
                            start=(k == 0),
                            stop=False,
                        )
                    nc.tensor.matmul(
                        out=ph[:],
                        lhsT=deg_sb[0:1, t * P : (t + 1) * P],
                        rhs=b_sb[:],
                        start=False,
                        stop=True,
                    )
                    # relu + layernorm
                    r_sb = work.tile([P, D], F32, tag="r")
                    nc.scalar.activation(
                        out=r_sb[:], in_=ph[:], func=mybir.ActivationFunctionType.Relu
                    )
                    st6 = stat.tile([P, 6], F32, tag="st6")
                    nc.vector.bn_stats(out=st6[:], in_=r_sb[:])
                    mv = stat.tile([P, 2], F32, tag="mv")
                    nc.vector.bn_aggr(out=mv[:], in_=st6[:])
                    rstd = stat.tile([P, 1], F32, tag="rstd")
                    nc.scalar.activation(
                        out=rstd[:], in_=mv[:, 1:2],
                        func=mybir.ActivationFunctionType.Sqrt, bias=eps_sb[:],
                    )
                    nc.vector.reciprocal(out=rstd[:], in_=rstd[:])
                    h_sb = work.tile([P, D], F32, tag="h")
                    nc.vector.tensor_scalar(
                        out=h_sb[:], in0=r_sb[:],
                        scalar1=mv[:, 0:1], scalar2=rstd[:],
                        op0=mybir.AluOpType.subtract, op1=mybir.AluOpType.mult,
                    )
                    if not trivial_ln:
                        nc.vector.tensor_mul(
                            out=h_sb[:], in0=h_sb[:], in1=gb_sb[:, 2 * gamma_i, :]
                        )
                        nc.vector.tensor_add(
                            out=h_sb[:], in0=h_sb[:], in1=gb_sb[:, 2 * gamma_i + 1, :]
                        )
                    if second:
                        h1t = work.tile([P, D], F32, tag="h1t")
                        nc.sync.dma_start(
                            out=h1t[:], in_=h1loc[t * P : (t + 1) * P, :]
                        )
                        nc.vector.tensor_add(out=h_sb[:], in0=h_sb[:], in1=h1t[:])
                    nc.sync.dma_start(
                        out=h_out_dram[t * P : (t + 1) * P, :], in_=h_sb[:]
                    )

            NM = meta["NM"]
            layer(embed_t[:, :], W1_sb, b1_sb, 0, h1loc, second=False)
            if debug:
                nc.sync.dma_start(out=dbg_h1[:, :], in_=h1loc[:, :])
                zgd = zg_pool.tile([P, D], F32, tag="zg")
                nc.gpsimd.indirect_dma_start(
                    out=zgd[:], out_offset=None, in_=embed_t[:, :],
                    in_offset=bass.IndirectOffsetOnAxis(ap=idx_sb[:, 0:1], axis=0),
                )
                nc.sync.dma_start(out=dbg_zg[:, 0:D], in_=zgd[:])
            nc.gpsimd.collective_compute(
                "AllGather",
                mybir.AluOpType.bypass,
                replica_groups=[list(range(C))],
                ins=[h1loc[0:NM, :].opt()],
                outs=[H1full[:, :].opt()],
            )
            if debug:
                nc.sync.dma_start(out=dbg_H1f[:, :], in_=H1full[:, :])
            layer(H1full[:, :], W2_sb, b2_sb, 1, embloc, second=True)

            # final owner-routed query gather
            for k in range(NFB * GB2):
                zq = zg_pool.tile([P, D], F32, tag="zq")
                nc.gpsimd.indirect_dma_start(
                    out=zq[:],
                    out_offset=None,
                    in_=embloc[:, :],
                    in_offset=bass.IndirectOffsetOnAxis(
                        ap=lidx_sb[:, k : k + 1], axis=0
                    ),
                )
                nc.sync.dma_start(
                    out=out_t[k * P : (k + 1) * P, :], in_=zq[:]
                )

    nc.compile()
    return nc


def kernel(**inputs):
    final, _, _, _ = _run(inputs, debug=False)
    return final


def _run(inputs, debug=False):
    x = np.asarray(inputs["x"])
    embed = np.asarray(inputs["embed"], dtype=np.float32)
    W1 = np.asarray(inputs["W1"], dtype=np.float32)
    b1 = np.asarray(inputs["b1"], dtype=np.float32)
    W2 = np.asarray(inputs["W2"], dtype=np.float32)
    b2 = np.asarray(inputs["b2"], dtype=np.float32)
    g1 = np.asarray(inputs["ln1_gamma"], dtype=np.float32)
    be1 = np.asarray(inputs["ln1_beta"], dtype=np.float32)
    g2 = np.asarray(inputs["ln2_gamma"], dtype=np.float32)
    be2 = np.asarray(inputs["ln2_beta"], dtype=np.float32)

    N, D = embed.shape
    B = x.shape[0]
    KD = D // P

    meta, arrays = _preprocess(
        x, embed, inputs["G_rows"], inputs["G_cols"], inputs["G_vals"]
    )
    trivial_ln = bool(
        np.all(g1 == 1) and np.all(be1 == 0) and np.all(g2 == 1) and np.all(be2 == 0)
    )
    nc = _build(meta, trivial_ln, debug=debug)

    W1r = np.ascontiguousarray(W1.reshape(KD, P, D).transpose(1, 0, 2))
    W2r = np.ascontiguousarray(W2.reshape(KD, P, D).transpose(1, 0, 2))
    in_maps = []
    for m in range(C):
        im = dict(
            embed=embed,
            S_img=arrays["S_imgs"][m],
            idx_all=arrays["idx_alls"][m],
            deg=arrays["degs"][m],
            W1r=W1r,
            W2r=W2r,
            b1=b1[None, :],
            b2=b2[None, :],
            lidx_all=arrays["lidx_alls"][m],
        )
        if not trivial_ln:
            im["gb"] = np.ascontiguousarray(
                np.stack(
                    [np.broadcast_to(v, (P, D)) for v in (g1, be1, g2, be2)], axis=1
                )
            )
        in_maps.append(im)

    res = run_bass_kernel_spmd(nc, in_maps, core_ids=list(range(C)))

    final = np.zeros((B, D), np.float32)
    for m in range(C):
        rows = meta["rows_list"][m]
        final[rows] = res.results[m]["out_packed"][: len(rows)]
    return final, res, meta, arrays


# revision 19
# speedup vs baseline: 1.0188x; 1.0188x over previous
"""Distributed GNN message-passing kernel for 8 Trainium2 NeuronCores.

Pipeline (mathematically equal to the reference, using G@(X@W) == (G@X)@W):
  u1 = G @ embed            (SpMM: gather + selection-matrix matmuls on PE)
  h1 = LN(relu(u1 @ W1 + deg*b1))
  H1 = AllGather(h1)        (the only collective)
  u2 = G @ H1
  h2 = LN(relu(u2 @ W2 + deg*b2))
  emb = h2 + h1
  out[b] = emb[x[b]-1] for valid queries (owner-routed, reassembled on host)

Sharding: nodes (rows of G / dest rows) are split contiguously across the 8
cores; [D,D] weights replicated; embed replicated (input staging, free);
h1 exchanged with one AllGather.
"""
import sys

sys.path.insert(0, "/opt/trn_rl_repo")

import numpy as np
from concourse import bacc, bass, mybir, tile
from concourse.bass_utils import run_bass_kernel_spmd
from concourse.masks import make_identity

C = 8          # cores
P = 128        # partitions
GB = 8         # SpMM gather groups per indirect-DMA call
GB2 = 8        # final-gather rows-per-partition per indirect-DMA call
EPS = 1e-5

F32 = mybir.dt.float32
I32 = mybir.dt.int32


def _preprocess(x, embed, G_rows, G_cols, G_vals):
    """Host-side index preprocessing. Returns per-core in_map arrays + meta."""
    N, D = embed.shape
    NM = N // C
    assert N % C == 0
    T = (NM + P - 1) // P
    x = np.asarray(x).astype(np.int64)
    G_rows = np.asarray(G_rows).astype(np.int64)
    G_cols = np.asarray(G_cols).astype(np.int64)
    G_vals = np.asarray(G_vals).astype(np.float32)

    HALF = 32768  # int16 index ceiling for dma_gather; cols >= HALF use table[HALF:]
    owner = G_rows // NM
    percore = []
    cnts = np.zeros((C, 2 * T), np.int64)  # per (tile, class) edge counts
    for m in range(C):
        sel = np.where(owner == m)[0]
        ld = G_rows[sel] - m * NM
        hi = (G_cols[sel] >= HALF).astype(np.int64)
        key = (ld // P) * 2 + hi
        order = np.argsort(key, kind="stable")
        sel, ld, hi, key = sel[order], ld[order], hi[order], key[order]
        percore.append((sel, ld, hi, key))
        cnts[m] = np.bincount(key, minlength=2 * T)

    ng_c = -(-cnts // P).max(axis=0).astype(np.int64)  # [2T] groups per (t, cls)
    glow = np.maximum(1, ng_c[0::2])
    ghigh = ng_c[1::2]
    gt = glow + ghigh                                  # [T] common group structure
    goff = np.concatenate([[0], np.cumsum(gt)]).astype(np.int64)
    Gtot = int(goff[-1])
    NB = Gtot  # kept for compat
    Gpad = Gtot

    gg = np.arange(Gtot * P) // P
    pp = np.arange(Gtot * P) % P

    # (tile, class) -> group range; class 0 groups first, then class 1
    cls_base = np.stack([goff[:-1], goff[:-1] + glow], axis=1)  # [T, 2]

    S_imgs, idx_alls, idx16_imgs, degs = [], [], [], []
    for m in range(C):
        sel, ld, hi, key = percore[m]
        tt = ld // P
        starts = np.concatenate([[0], np.cumsum(cnts[m])])
        r = np.arange(len(ld)) - starts[key]
        slot = cls_base[tt, hi] * P + r
        cols_pad = np.zeros(Gpad * P, np.int32)
        vals_pad = np.zeros(Gtot * P, np.float32)
        dof_pad = np.zeros(Gtot * P, np.int64)
        cols_pad[slot] = G_cols[sel]
        vals_pad[slot] = G_vals[sel]
        dof_pad[slot] = ld - tt * P
        # high-class groups address table[HALF:]
        adj = cols_pad.copy()
        is_high_slot = np.zeros(Gtot, np.bool_)
        for t in range(T):
            is_high_slot[cls_base[t, 1] : goff[t + 1]] = True
        adj = adj - (np.repeat(is_high_slot, P) * HALF)
        assert adj.min() >= 0 and adj.max() < HALF
        idx16 = adj.astype(np.int16).reshape(-1, 16).T  # [16, Gtot*8]
        idx16_imgs.append(np.ascontiguousarray(np.tile(idx16, (8, 1))))
        S = np.zeros((Gtot, P, P), np.float32)
        S[gg, pp, dof_pad] = vals_pad
        S_imgs.append(np.ascontiguousarray(S.transpose(1, 0, 2).reshape(P, Gtot * P)))
        idx_alls.append(np.ascontiguousarray(cols_pad.reshape(Gpad, P).T))
        deg = np.zeros(T * P, np.float32)
        deg[:NM] = np.bincount(ld, weights=G_vals[sel], minlength=NM)
        degs.append(deg[None, :])

    # final query routing
    selq = (x >= 1) & (x < N + 1)
    idx = np.clip(x - 1, 0, N - 1)
    qowner = idx // NM
    rows_list, lidx_list = [], []
    for m in range(C):
        rows = np.where(selq & (qowner == m))[0]
        rows_list.append(rows)
        lidx_list.append((idx[rows] - m * NM).astype(np.int32))
    max_cnt = max(len(r) for r in rows_list)
    NFB = max(1, -(-max_cnt // (P * GB2)))
    cnt_pad = NFB * P * GB2
    lidx_alls, lidx16_imgs = [], []
    for m in range(C):
        lp = np.zeros(cnt_pad, np.int32)
        lp[: len(lidx_list[m])] = lidx_list[m]
        lidx_alls.append(np.ascontiguousarray(lp.reshape(NFB * GB2, P).T))
        l16 = lp.astype(np.int16).reshape(-1, 16).T  # [16, cnt_pad/16]
        lidx16_imgs.append(np.ascontiguousarray(np.tile(l16, (8, 1))))

    meta = dict(
        N=N, D=D, NM=NM, T=T, gt=gt, goff=goff, Gtot=Gtot, NB=NB, NFB=NFB,
        cnt_pad=cnt_pad, rows_list=rows_list,
        glow=glow, ghigh=ghigh, cls_base=cls_base, HALF=HALF,
    )
    arrays = dict(S_imgs=S_imgs, idx_alls=idx_alls, degs=degs,
                  lidx_alls=lidx_alls, idx16_imgs=idx16_imgs,
                  lidx16_imgs=lidx16_imgs)
    return meta, arrays


def _build(meta, trivial_ln, debug=False):
    """Build the SPMD Bass program (identical across cores)."""
    N, D, T = meta["N"], meta["D"], meta["T"]
    gt, goff, Gtot, NB, NFB = (
        meta["gt"], meta["goff"], meta["Gtot"], meta["NB"], meta["NFB"])
    cnt_pad = meta["cnt_pad"]
    KD = D // P  # contraction chunks for dense matmul (4 for D=512)

    nc = bacc.Bacc(None, num_devices=C)

    # --- I/O ---
    embed_t = nc.dram_tensor("embed", [N, D], F32, kind="ExternalInput")
    S_t = nc.dram_tensor("S_img", [P, Gtot * P], F32, kind="ExternalInput")
    idx_t = nc.dram_tensor("idx_all", [P, NB * GB], I32, kind="ExternalInput")
    deg_t = nc.dram_tensor("deg", [1, T * P], F32, kind="ExternalInput")
    W1_t = nc.dram_tensor("W1r", [P, KD, D], F32, kind="ExternalInput")
    W2_t = nc.dram_tensor("W2r", [P, KD, D], F32, kind="ExternalInput")
    b1_t = nc.dram_tensor("b1", [1, D], F32, kind="ExternalInput")
    b2_t = nc.dram_tensor("b2", [1, D], F32, kind="ExternalInput")
    lidx_t = nc.dram_tensor("lidx_all", [P, NFB * GB2], I32, kind="ExternalInput")
    if not trivial_ln:
        gb_t = nc.dram_tensor("gb", [P, 4, D], F32, kind="ExternalInput")
    out_t = nc.dram_tensor("out_packed", [cnt_pad, D], F32, kind="ExternalOutput")
    if debug:
        dbg_u1 = nc.dram_tensor("dbg_u1", [T * P, D], F32, kind="ExternalOutput")
        dbg_h1 = nc.dram_tensor("dbg_h1", [T * P, D], F32, kind="ExternalOutput")
        dbg_H1f = nc.dram_tensor("dbg_H1f", [N, D], F32, kind="ExternalOutput")
        dbg_zg = nc.dram_tensor("dbg_zg", [P, GB * D], F32, kind="ExternalOutput")

    with tile.TileContext(nc) as tc:
        with (
            tc.tile_pool(name="zg", bufs=24) as zg_pool,
            tc.tile_pool(name="sp", bufs=3) as s_pool,
            tc.tile_pool(name="small", bufs=1) as small,
            tc.tile_pool(name="work", bufs=4) as work,
            tc.tile_pool(name="stat", bufs=6) as stat,
            tc.tile_pool(name="pu", bufs=2, space="PSUM") as psum_u,
            tc.tile_pool(name="pt", bufs=2, space="PSUM") as psum_t,
            tc.tile_pool(name="ph", bufs=2, space="PSUM") as psum_h,
            tc.tile_pool(name="dram", bufs=1, space="DRAM") as dram,
        ):
            # --- internal DRAM ---
            h1loc = dram.tile([T * P, D], F32)
            H1full = dram.tile([N, D], F32)
            embloc = dram.tile([T * P, D], F32)

            # --- constants ---
            W1_sb = small.tile([P, KD, D], F32)
            nc.sync.dma_start(out=W1_sb[:], in_=W1_t[:])
            W2_sb = small.tile([P, KD, D], F32)
            nc.sync.dma_start(out=W2_sb[:], in_=W2_t[:])
            b1_sb = small.tile([1, D], F32)
            nc.sync.dma_start(out=b1_sb[:], in_=b1_t[:])
            b2_sb = small.tile([1, D], F32)
            nc.sync.dma_start(out=b2_sb[:], in_=b2_t[:])
            deg_sb = small.tile([1, T * P], F32)
            nc.sync.dma_start(out=deg_sb[:], in_=deg_t[:])
            idx_sb = small.tile([P, NB * GB], I32)
            nc.sync.dma_start(out=idx_sb[:], in_=idx_t[:])
            lidx_sb = small.tile([P, NFB * GB2], I32)
            nc.sync.dma_start(out=lidx_sb[:], in_=lidx_t[:])
            ident = small.tile([P, P], F32)
            make_identity(nc, ident[:])
            eps_sb = small.tile([P, 1], F32)
            nc.vector.memset(eps_sb[:], EPS)
            if not trivial_ln:
                gb_sb = small.tile([P, 4, D], F32)
                nc.sync.dma_start(out=gb_sb[:], in_=gb_t[:])

            def layer(table_ap, W_sb, b_sb, gamma_i, h_out_dram, second):
                # gather producers: one [P,1]-indexed indirect DMA per group
                # (HW honors exactly one index per partition per call)
                zg_tiles = []
                for g in range(Gtot):
                    zg = zg_pool.tile([P, D], F32, tag="zg")
                    nc.gpsimd.indirect_dma_start(
                        out=zg[:],
                        out_offset=None,
                        in_=table_ap,
                        in_offset=bass.IndirectOffsetOnAxis(
                            ap=idx_sb[:, g : g + 1], axis=0
                        ),
                    )
                    zg_tiles.append(zg)

                for t in range(T):
                    g0, g1 = int(goff[t]), int(goff[t + 1])
                    ngt = g1 - g0
                    s_sb = s_pool.tile([P, ngt * P], F32, tag="s")
                    nc.sync.dma_start(
                        out=s_sb[:], in_=S_t[:, g0 * P : g1 * P]
                    )
                    pu = psum_u.tile([P, D], F32)
                    for gl in range(ngt):
                        nc.tensor.matmul(
                            out=pu[:],
                            lhsT=s_sb[:, gl * P : (gl + 1) * P],
                            rhs=zg_tiles[g0 + gl][:],
                            start=(gl == 0),
                            stop=(gl == ngt - 1),
                        )
                    u_sb = work.tile([P, D], F32, tag="u")
                    nc.vector.tensor_copy(out=u_sb[:], in_=pu[:])
                    if debug and not second:
                        nc.sync.dma_start(
                            out=dbg_u1[t * P : (t + 1) * P, :], in_=u_sb[:]
                        )
                    pt = psum_t.tile([P, D], F32)
                    for k in range(KD):
                        nc.tensor.transpose(
                            out=pt[:, k * P : (k + 1) * P],
                            in_=u_sb[:, k * P : (k + 1) * P],
                            identity=ident[:],
                        )
                    uT_sb = work.tile([P, D], F32, tag="ut")
                    nc.vector.tensor_copy(out=uT_sb[:], in_=pt[:])
                    ph = psum_h.tile([P, D], F32)
                    for k in range(KD):
                        nc.tensor.matmul(
                            out=ph[:],
                            lhsT=uT_sb[:, k * P : (k + 1) * P],
                            rhs=W_sb[:, k, :],
                            start=(k == 0),
                            stop=False,
                        )
                    nc.tensor.matmul(
                        out=ph[:],
                        lhsT=deg_sb[0:1, t * P : (t + 1) * P],
                        rhs=b_sb[:],
                        start=False,
                        stop=True,
                    )
                    # relu + layernorm
                    r_sb = work.tile([P, D], F32, tag="r")
                    nc.scalar.activation(
                        out=r_sb[:], in_=ph[:], func=mybir.ActivationFunctionType.Relu
                    )
                    st6 = stat.tile([P, 6], F32, tag="st6")
                    nc.vector.bn_stats(out=st6[:], in_=r_sb[:])
                    mv = stat.tile([P, 2], F32, tag="mv")
                    nc.vector.bn_aggr(out=mv[:], in_=st6[:])
                    rstd = stat.tile([P, 1], F32, tag="rstd")
                    nc.scalar.activation(
                        out=rstd[:], in_=mv[:, 1:2],
                        func=mybir.ActivationFunctionType.Sqrt, bias=eps_sb[:],
                    )
                    nc.vector.reciprocal(out=rstd[:], in_=rstd[:])
                    h_sb = work.tile([P, D], F32, tag="h")
                    nc.vector.tensor_scalar(
                        out=h_sb[:], in0=r_sb[:],
                        scalar1=mv[:, 0:1], scalar2=rstd[:],
                        op0=mybir.AluOpType.subtract, op1=mybir.AluOpType.mult,
                    )
                    if not trivial_ln:
                        nc.vector.tensor_mul(
                            out=h_sb[:], in0=h_sb[:], in1=gb_sb[:, 2 * gamma_i, :]
                        )
                        nc.vector.tensor_add(
                            out=h_sb[:], in0=h_sb[:], in1=gb_sb[:, 2 * gamma_i + 1, :]
                        )
                    if second:
                        h1t = work.tile([P, D], F32, tag="h1t")
                        nc.sync.dma_start(
                            out=h1t[:], in_=h1loc[t * P : (t + 1) * P, :]
                        )
                        nc.vector.tensor_add(out=h_sb[:], in0=h_sb[:], in1=h1t[:])
                    nc.sync.dma_start(
                        out=h_out_dram[t * P : (t + 1) * P, :], in_=h_sb[:]
                    )

            NM = meta["NM"]
            layer(embed_t[:, :], W1_sb, b1_sb, 0, h1loc, second=False)
            if debug:
                nc.sync.dma_start(out=dbg_h1[:, :], in_=h1loc[:, :])
                zgd = zg_pool.tile([P, D], F32, tag="zg")
                nc.gpsimd.indirect_dma_start(
                    out=zgd[:], out_offset=None, in_=embed_t[:, :],
                    in_offset=bass.IndirectOffsetOnAxis(ap=idx_sb[:, 0:1], axis=0),
                )
                nc.sync.dma_start(out=dbg_zg[:, 0:D], in_=zgd[:])
            nc.gpsimd.collective_compute(
                "AllGather",
                mybir.AluOpType.bypass,
                replica_groups=[list(range(C))],
                ins=[h1loc[0:NM, :].opt()],
                outs=[H1full[:, :].opt()],
            )
            if debug:
                nc.sync.dma_start(out=dbg_H1f[:, :], in_=H1full[:, :])
            layer(H1full[:, :], W2_sb, b2_sb, 1, embloc, second=True)

            # final owner-routed query gather
            for k in range(NFB * GB2):
                zq = zg_pool.tile([P, D], F32, tag="zq")
                nc.gpsimd.indirect_dma_start(
                    out=zq[:],
                    out_offset=None,
                    in_=embloc[:, :],
                    in_offset=bass.IndirectOffsetOnAxis(
                        ap=lidx_sb[:, k : k + 1], axis=0
                    ),
                )
                nc.sync.dma_start(
                    out=out_t[k * P : (k + 1) * P, :], in_=zq[:]
                )

    nc.compile()
    return nc


def kernel(**inputs):
    final, _, _, _ = _run(inputs, debug=False)
    return final


def _run(inputs, debug=False):
    x = np.asarray(inputs["x"])
    embed = np.asarray(inputs["embed"], dtype=np.float32)
    W1 = np.asarray(inputs["W1"], dtype=np.float32)
    b1 = np.asarray(inputs["b1"], dtype=np.float32)
    W2 = np.asarray(inputs["W2"], dtype=np.float32)
    b2 = np.asarray(inputs["b2"], dtype=np.float32)
    g1 = np.asarray(inputs["ln1_gamma"], dtype=np.float32)
    be1 = np.asarray(inputs["ln1_beta"], dtype=np.float32)
    g2 = np.asarray(inputs["ln2_gamma"], dtype=np.float32)
    be2 = np.asarray(inputs["ln2_beta"], dtype=np.float32)

    N, D = embed.shape
    B = x.shape[0]
    KD = D // P

    meta, arrays = _preprocess(
        x, embed, inputs["G_rows"], inputs["G_cols"], inputs["G_vals"]
    )
    trivial_ln = bool(
        np.all(g1 == 1) and np.all(be1 == 0) and np.all(g2 == 1) and np.all(be2 == 0)
    )
    nc = _build(meta, trivial_ln, debug=debug)

    W1r = np.ascontiguousarray(W1.reshape(KD, P, D).transpose(1, 0, 2))
    W2r = np.ascontiguousarray(W2.reshape(KD, P, D).transpose(1, 0, 2))
    in_maps = []
    for m in range(C):
        im = dict(
            embed=embed,
            S_img=arrays["S_imgs"][m],
            idx_all=arrays["idx_alls"][m],
            deg=arrays["degs"][m],
            W1r=W1r,
            W2r=W2r,
            b1=b1[None, :],
            b2=b2[None, :],
            lidx_all=arrays["lidx_alls"][m],
        )
        if not trivial_ln:
            im["gb"] = np.ascontiguousarray(
                np.stack(
                    [np.broadcast_to(v, (P, D)) for v in (g1, be1, g2, be2)], axis=1
                )
            )
        in_maps.append(im)

    res = run_bass_kernel_spmd(nc, in_maps, core_ids=list(range(C)))

    final = np.zeros((B, D), np.float32)
    for m in range(C):
        rows = meta["rows_list"][m]
        final[rows] = res.results[m]["out_packed"][: len(rows)]
    return final, res, meta, arrays
